# revision 7
# baseline (speedup 1.0000x reference)
"""Trainium2 Bass kernel for nn_NodeConv (GNN message passing).

Strategy (8 NeuronCores, data-parallel, no collectives):
  - Nodes are partitioned into 8 contiguous ranges; every edge is routed to
    the core that owns its *destination* node, so the segment-sum is fully
    local to each core.  MLP weights are replicated.
  - On the host, each core's nodes are sorted by in-degree and packed into
    groups of 128.  Edge features are laid out in an ELL-style slab
    [128 nodes x D_g chunks x 128 feat] (D_g = max degree in group, ~1-2%
    padding thanks to the degree sort).
  - On the device the segment-sum is performed by the TensorEngine:
    for each chunk, matmul(lhsT=chunk, rhs=I128) accumulates chunk^T into
    PSUM, yielding the per-group message matrix *feature-major* with zero
    per-edge elementwise work.  Edge features travel as fp16 (~5e-4 rel err).
  - The MLP runs feature-major (weights as lhsT), the last layer uses the
    activations as lhsT which transposes the result back to node-major for
    the GroupNorm + residual, and the output is DMA'd node-major.
"""

import sys

sys.path.insert(0, "/opt/trn_rl_repo")

import numpy as np

import concourse.bass as bass
import concourse.bacc as bacc
import concourse.tile as tile
from concourse import mybir
from concourse.bass_utils import run_bass_kernel_spmd

P = 128
N_CORES = 8
SG = 4          # groups per supergroup (MLP batch = 512 nodes)
EPS = 1e-5

F16 = mybir.dt.float16
F32 = mybir.dt.float32
AF = mybir.ActivationFunctionType
ALU = mybir.AluOpType


# --------------------------------------------------------------------------
# Host-side sharding / layout
# --------------------------------------------------------------------------

def _host_prep(x, e, edge_index):
    """Shard nodes/edges across cores and build per-core ELL slabs."""
    n_nodes = x.shape[0]
    npc = -(-n_nodes // N_CORES)              # nodes per core (ceil)
    dst = np.asarray(edge_index[1]).astype(np.int64)
    e16 = np.ascontiguousarray(e, dtype=np.float16)
    e16z = np.vstack([e16, np.zeros((1, e16.shape[1]), np.float16)])
    zero_row = e16.shape[0]

    cores = []
    for c in range(N_CORES):
        lo, hi = c * npc, min((c + 1) * npc, n_nodes)
        sel = np.nonzero((dst >= lo) & (dst < hi))[0]
        ldst = (dst[sel] - lo).astype(np.int64)
        n_real = hi - lo
        deg = np.bincount(ldst, minlength=npc)
        order = np.argsort(-deg, kind="stable")       # all npc local ids
        # edges sorted by local dst; esort[k] = global edge row
        order_e = np.argsort(ldst, kind="stable")
        esort = sel[order_e]
        starts = np.zeros(npc + 1, np.int64)
        np.cumsum(deg, out=starts[1:])
        cores.append(dict(lo=lo, n_real=n_real, deg=deg, order=order,
                          esort=esort, starts=starts))

    # canonical group schedule, shared by all cores
    ngrp = -(-npc // P)
    ngrp = -(-ngrp // SG) * SG                # round up to supergroup multiple
    npc_pad = ngrp * P
    d_list = np.ones(ngrp, np.int64)
    for c in cores:
        degs = np.zeros(npc_pad, np.int64)
        degs[:npc] = c["deg"][c["order"]]
        dg = degs.reshape(ngrp, P).max(axis=1)
        d_list = np.maximum(d_list, dg)
    d_list = np.maximum(d_list, 1)
    offs = np.zeros(ngrp + 1, np.int64)
    np.cumsum(d_list * P * P, out=offs[1:])
    tot = int(offs[-1])

    in_maps = []
    for c in cores:
        slab = np.zeros(tot, np.float16)
        order = c["order"]
        deg, starts, esort = c["deg"], c["starts"], c["esort"]
        for g in range(ngrp):
            d = int(d_list[g])
            nid = order[g * P:(g + 1) * P]            # may be short at tail
            if len(nid) == 0:
                continue
            degs_g = deg[nid][:, None]                 # [p, 1]
            ks = np.arange(d)[None, :]                 # [1, d]
            valid = ks < degs_g
            pos = starts[nid][:, None] + ks
            rows = np.where(valid, esort[np.minimum(pos, len(esort) - 1)],
                            zero_row)
            block = e16z[rows]                          # [p, d, 128]
            dst_view = slab[offs[g]:offs[g + 1]].reshape(-1, d * P)
            dst_view[:len(nid)] = block.reshape(len(nid), d * P)
        in_maps.append(dict(e_ell=slab))

    meta = dict(npc=npc, ngrp=ngrp, npc_pad=npc_pad,
                d_list=d_list, offs=offs, tot=tot, cores=cores)
    return in_maps, meta


def _host_prep_x(x, meta):
    """Build per-core x buffers (node-major + feature-major)."""
    npc, npc_pad = meta["npc"], meta["npc_pad"]
    out = []
    for c in meta["cores"]:
        xp = np.zeros((npc_pad, P), np.float32)
        xr = np.asarray(x[c["lo"]:c["lo"] + c["n_real"]], np.float32)
        perm = c["order"]
        # rows beyond n_real in `perm` index nodes that don't exist for the
        # tail core; keep them zero.
        valid = perm < c["n_real"]
        xp[np.nonzero(valid)[0]] = xr[perm[valid]]
        xt = np.ascontiguousarray(xp.T)
        out.append((xp, xt))
    return out


# --------------------------------------------------------------------------
# Device program
# --------------------------------------------------------------------------

def _build_program(meta, flags):
    ngrp, npc_pad = meta["ngrp"], meta["npc_pad"]
    d_list, offs, tot = meta["d_list"], meta["offs"], meta["tot"]
    use_bo = flags["use_bo"]
    use_gn = flags["use_gn"]

    nc = bacc.Bacc("TRN2", target_bir_lowering=False, debug=False)

    e_ell = nc.dram_tensor("e_ell", [tot], F16, kind="ExternalInput").ap()
    xT_d = nc.dram_tensor("xT", [P, npc_pad], F32, kind="ExternalInput").ap()
    x_d = nc.dram_tensor("x_nm", [npc_pad, P], F32, kind="ExternalInput").ap()
    w0x_d = nc.dram_tensor("W0x", [P, P], F32, kind="ExternalInput").ap()
    w0m_d = nc.dram_tensor("W0m", [P, P], F32, kind="ExternalInput").ap()
    wh0_d = nc.dram_tensor("Wh0", [P, P], F32, kind="ExternalInput").ap()
    wh1_d = nc.dram_tensor("Wh1", [P, P], F32, kind="ExternalInput").ap()
    wo_d = nc.dram_tensor("Wo", [P, P], F32, kind="ExternalInput").ap()
    b0_d = nc.dram_tensor("b0", [P, 1], F32, kind="ExternalInput").ap()
    bh0_d = nc.dram_tensor("bh0", [P, 1], F32, kind="ExternalInput").ap()
    bh1_d = nc.dram_tensor("bh1", [P, 1], F32, kind="ExternalInput").ap()
    i_d = nc.dram_tensor("I16", [P, P], F16, kind="ExternalInput").ap()
    if use_bo:
        bo_d = nc.dram_tensor("bo_b", [P, SG * P], F32,
                              kind="ExternalInput").ap()
    if use_gn:
        gnw_d = nc.dram_tensor("gnw_b", [P, SG * P], F32,
                               kind="ExternalInput").ap()
        gnb_d = nc.dram_tensor("gnb_b", [P, SG * P], F32,
                               kind="ExternalInput").ap()
    out_d = nc.dram_tensor("out", [npc_pad, P], F32, kind="ExternalOutput").ap()

    nsg = ngrp // SG
    W = SG * P

    with tile.TileContext(nc) as tc:
        with (
            tc.tile_pool(name="const", bufs=1) as cpool,
            tc.tile_pool(name="slab", bufs=6) as spool,
            tc.tile_pool(name="io", bufs=3) as iopool,
            tc.tile_pool(name="act", bufs=2) as apool,
            tc.tile_pool(name="gn", bufs=2) as gpool,
            tc.tile_pool(name="stat", bufs=2) as tpool,
            tc.tile_pool(name="pmsg", bufs=2, space="PSUM") as pmsg,
            tc.tile_pool(name="pmlp", bufs=3, space="PSUM") as pmlp,
            tc.tile_pool(name="pout", bufs=2, space="PSUM") as pout,
        ):
            ident = cpool.tile([P, P], F16)
            nc.sync.dma_start(ident[:], i_d[:])
            eps_t = cpool.tile([P, 1], F32)
            nc.vector.memset(eps_t[:], EPS)
            w0x = cpool.tile([P, P], F32)
            nc.sync.dma_start(w0x[:], w0x_d[:])
            w0m = cpool.tile([P, P], F32)
            nc.sync.dma_start(w0m[:], w0m_d[:])
            wh0 = cpool.tile([P, P], F32)
            nc.sync.dma_start(wh0[:], wh0_d[:])
            wh1 = cpool.tile([P, P], F32)
            nc.sync.dma_start(wh1[:], wh1_d[:])
            wo = cpool.tile([P, P], F32)
            nc.sync.dma_start(wo[:], wo_d[:])
            b0 = cpool.tile([P, 1], F32)
            nc.sync.dma_start(b0[:], b0_d[:])
            bh0 = cpool.tile([P, 1], F32)
            nc.sync.dma_start(bh0[:], bh0_d[:])
            bh1 = cpool.tile([P, 1], F32)
            nc.sync.dma_start(bh1[:], bh1_d[:])
            if use_bo:
                bo_b = cpool.tile([P, W], F32)
                nc.sync.dma_start(bo_b[:], bo_d[:])
            if use_gn:
                gnw_b = cpool.tile([P, W], F32)
                nc.sync.dma_start(gnw_b[:], gnw_d[:])
                gnb_b = cpool.tile([P, W], F32)
                nc.sync.dma_start(gnb_b[:], gnb_d[:])

            for s in range(nsg):
                g0 = s * SG
                # ---- message scatter: msgT[feat, node] ----
                msg_ps = pmsg.tile([P, W], F32, tag="msg")
                for gi in range(SG):
                    g = g0 + gi
                    d = int(d_list[g])
                    slab = spool.tile([P, d * P], F16, tag="slab")
                    nc.sync.dma_start(
                        slab[:],
                        e_ell[int(offs[g]):int(offs[g + 1])]
                        .rearrange("(p w) -> p w", p=P),
                    )
                    dst = msg_ps[:, gi * P:(gi + 1) * P]
                    for k in range(d):
                        nc.tensor.matmul(
                            dst,
                            lhsT=slab[:, k * P:(k + 1) * P],
                            rhs=ident[:],
                            start=(k == 0),
                            stop=(k == d - 1),
                        )
                msg_s = apool.tile([P, W], F32, tag="msg_s")
                nc.scalar.copy(msg_s[:], msg_ps[:])

                # ---- MLP (feature-major) ----
                xt = iopool.tile([P, W], F32, tag="xT")
                nc.sync.dma_start(xt[:], xT_d[:, s * W:(s + 1) * W])

                h_ps = pmlp.tile([P, W], F32, tag="mlp")
                nc.tensor.matmul(h_ps[:], lhsT=w0x[:], rhs=xt[:],
                                 start=True, stop=False)
                nc.tensor.matmul(h_ps[:], lhsT=w0m[:], rhs=msg_s[:],
                                 start=False, stop=True)
                h1 = apool.tile([P, W], F32, tag="h")
                nc.scalar.activation(h1[:], h_ps[:], AF.Relu, bias=b0[:, 0:1])

                h_ps2 = pmlp.tile([P, W], F32, tag="mlp")
                nc.tensor.matmul(h_ps2[:], lhsT=wh0[:], rhs=h1[:],
                                 start=True, stop=True)
                h2 = apool.tile([P, W], F32, tag="h")
                nc.scalar.activation(h2[:], h_ps2[:], AF.Relu, bias=bh0[:, 0:1])

                h_ps3 = pmlp.tile([P, W], F32, tag="mlp")
                nc.tensor.matmul(h_ps3[:], lhsT=wh1[:], rhs=h2[:],
                                 start=True, stop=True)
                h3 = apool.tile([P, W], F32, tag="h")
                nc.scalar.activation(h3[:], h_ps3[:], AF.Relu, bias=bh1[:, 0:1])

                # ---- output layer, node-major out[node, ch] ----
                o_ps = pout.tile([P, W], F32, tag="o")
                for gi in range(SG):
                    nc.tensor.matmul(
                        o_ps[:, gi * P:(gi + 1) * P],
                        lhsT=h3[:, gi * P:(gi + 1) * P],
                        rhs=wo[:],
                        start=True, stop=True,
                    )

                # ---- GroupNorm(1, C) + residual ----
                if use_bo:
                    basis = gpool.tile([P, W], F32, tag="basis")
                    nc.vector.tensor_add(basis[:], o_ps[:], bo_b[:])
                else:
                    basis = o_ps
                b3 = basis[:].rearrange("p (g c) -> p g c", c=P)
                s1 = tpool.tile([P, SG], F32, tag="s1")
                nc.vector.tensor_reduce(s1[:], b3, axis=mybir.AxisListType.X,
                                        op=ALU.add)
                sq = gpool.tile([P, W], F32, tag="sq")
                nc.scalar.square(sq[:], basis[:])
                s2 = tpool.tile([P, SG], F32, tag="s2")
                nc.vector.tensor_reduce(
                    s2[:], sq[:].rearrange("p (g c) -> p g c", c=P),
                    axis=mybir.AxisListType.X, op=ALU.add)
                mu = tpool.tile([P, SG], F32, tag="mu")
                nc.vector.tensor_scalar_mul(mu[:], s1[:], 1.0 / P)
                ex2 = tpool.tile([P, SG], F32, tag="ex2")
                nc.vector.tensor_scalar_mul(ex2[:], s2[:], 1.0 / P)
                var = tpool.tile([P, SG], F32, tag="var")
                nc.vector.tensor_tensor(var[:], mu[:], mu[:], op=ALU.mult)
                nc.vector.tensor_tensor(var[:], ex2[:], var[:],
                                        op=ALU.subtract)
                sd = tpool.tile([P, SG], F32, tag="sd")
                nc.scalar.activation(sd[:], var[:], AF.Sqrt,
                                     bias=eps_t[:, 0:1])
                rinv = tpool.tile([P, SG], F32, tag="rinv")
                nc.vector.reciprocal(rinv[:], sd[:])
                negms = tpool.tile([P, SG], F32, tag="negms")
                nc.vector.tensor_tensor(negms[:], mu[:], rinv[:], op=ALU.mult)
                nc.vector.tensor_scalar_mul(negms[:], negms[:], -1.0)

                for gi in range(SG):
                    g = g0 + gi
                    xt_res = iopool.tile([P, P], F32, tag="xres")
                    nc.sync.dma_start(xt_res[:], x_d[g * P:(g + 1) * P, :])
                    u = gpool.tile([P, P], F32, tag="u")
                    nc.vector.tensor_scalar(
                        u[:], basis[:, gi * P:(gi + 1) * P],
                        scalar1=rinv[:, gi:gi + 1],
                        scalar2=negms[:, gi:gi + 1],
                        op0=ALU.mult, op1=ALU.add)
                    if use_gn:
                        nc.vector.tensor_tensor(
                            u[:], u[:], gnw_b[:, gi * P:(gi + 1) * P],
                            op=ALU.mult)
                        nc.vector.tensor_tensor(
                            u[:], u[:], gnb_b[:, gi * P:(gi + 1) * P],
                            op=ALU.add)
                    ot = gpool.tile([P, P], F32, tag="ot")
                    nc.vector.tensor_add(ot[:], u[:], xt_res[:])
                    nc.sync.dma_start(out_d[g * P:(g + 1) * P, :], ot[:])

    return nc


# --------------------------------------------------------------------------
# Entry point
# --------------------------------------------------------------------------

def _run(inputs, trace=False):
    x = np.asarray(inputs["x"], np.float32)
    e = np.asarray(inputs["e"], np.float32)
    edge_index = np.asarray(inputs["edge_index"])
    W0 = np.asarray(inputs["W0"], np.float32)
    b0 = np.asarray(inputs["b0"], np.float32)
    Wh = np.asarray(inputs["Wh"], np.float32)
    bh = np.asarray(inputs["bh"], np.float32)
    Wo = np.asarray(inputs["Wo"], np.float32)
    bo = np.asarray(inputs["bo"], np.float32)
    gn_w = np.asarray(inputs["gn_w"], np.float32)
    gn_b = np.asarray(inputs["gn_b"], np.float32)

    import time as _time
    _t0 = _time.monotonic()
    in_maps, meta = _host_prep(x, e, edge_index)
    xbufs = _host_prep_x(x, meta)
    print(f"[kernel] host prep {_time.monotonic()-_t0:.1f}s", flush=True)

    flags = dict(use_bo=bool(np.any(bo != 0.0)),
                 use_gn=bool(np.any(gn_w != 1.0) or np.any(gn_b != 0.0)))

    consts = dict(
        W0x=np.ascontiguousarray(W0[:P]),
        W0m=np.ascontiguousarray(W0[P:]),
        Wh0=np.ascontiguousarray(Wh[0]),
        Wh1=np.ascontiguousarray(Wh[1]),
        Wo=np.ascontiguousarray(Wo),
        b0=b0.reshape(P, 1).copy(),
        bh0=bh[0].reshape(P, 1).copy(),
        bh1=bh[1].reshape(P, 1).copy(),
        I16=np.eye(P, dtype=np.float16),
    )
    if flags["use_bo"]:
        consts["bo_b"] = np.tile(bo[None, :], (P, SG)).astype(np.float32)
    if flags["use_gn"]:
        consts["gnw_b"] = np.tile(gn_w[None, :], (P, SG)).astype(np.float32)
        consts["gnb_b"] = np.tile(gn_b[None, :], (P, SG)).astype(np.float32)

    _t0 = _time.monotonic()
    nc = _build_program(meta, flags)
    print(f"[kernel] build {_time.monotonic()-_t0:.1f}s", flush=True)
    _t0 = _time.monotonic()
    nc.compile()
    print(f"[kernel] bacc compile {_time.monotonic()-_t0:.1f}s", flush=True)
    _t0 = _time.monotonic()

    full_maps = []
    for c in range(N_CORES):
        m = dict(in_maps[c])
        m["x_nm"], m["xT"] = xbufs[c]
        m.update(consts)
        full_maps.append(m)

    res = run_bass_kernel_spmd(nc, full_maps, list(range(N_CORES)),
                               trace=trace)
    print(f"[kernel] spmd run {_time.monotonic()-_t0:.1f}s", flush=True)

    n_nodes = x.shape[0]
    out = np.empty((n_nodes, P), np.float32)
    npc = meta["npc"]
    for c, cinfo in enumerate(meta["cores"]):
        oc = res.results[c]["out"]
        perm = cinfo["order"]
        valid = perm < cinfo["n_real"]
        rows = np.nonzero(valid)[0]
        out[cinfo["lo"] + perm[valid]] = oc[rows]
    return out, res


def kernel(**inputs):
    out, _ = _run(inputs, trace=False)
    return out


# revision 16
# speedup vs baseline: 1.3201x; 1.3201x over previous
"""Trainium2 Bass kernel for nn_NodeConv (GNN message passing).

Strategy (8 NeuronCores, data-parallel, no collectives):
  - Nodes are partitioned into 8 contiguous ranges; every edge is routed to
    the core that owns its *destination* node, so the segment-sum is fully
    local to each core.  MLP weights are replicated.
  - On the host, each core's nodes are sorted by in-degree and packed into
    groups of 128.  Edge features are laid out in an ELL-style slab
    [128 nodes x D_g chunks x 128 feat] (D_g = max degree in group, ~1-2%
    padding thanks to the degree sort).
  - On the device the segment-sum is performed by the TensorEngine:
    for each chunk, matmul(lhsT=chunk, rhs=I128) accumulates chunk^T into
    PSUM, yielding the per-group message matrix *feature-major* with zero
    per-edge elementwise work.  Edge features travel as fp16 (~5e-4 rel err).
  - The MLP runs feature-major (weights as lhsT), the last layer uses the
    activations as lhsT which transposes the result back to node-major for
    the GroupNorm + residual, and the output is DMA'd node-major.
"""

import sys

sys.path.insert(0, "/opt/trn_rl_repo")

import numpy as np

import concourse.bass as bass
import concourse.bacc as bacc
import concourse.tile as tile
from concourse import mybir
from concourse.bass_utils import run_bass_kernel_spmd

P = 128
N_CORES = 8
SG = 4          # groups per supergroup (MLP batch = 512 nodes)
EPS = 1e-5

F16 = mybir.dt.float16
F32 = mybir.dt.float32
AF = mybir.ActivationFunctionType
ALU = mybir.AluOpType


# --------------------------------------------------------------------------
# Host-side sharding / layout
# --------------------------------------------------------------------------

def _host_prep(x, e, edge_index):
    """Shard nodes/edges across cores and build per-core ELL slabs."""
    n_nodes = x.shape[0]
    npc = -(-n_nodes // N_CORES)              # nodes per core (ceil)
    dst = np.asarray(edge_index[1]).astype(np.int64)
    e16 = np.ascontiguousarray(e, dtype=np.float16)
    e16z = np.vstack([e16, np.zeros((1, e16.shape[1]), np.float16)])
    zero_row = e16.shape[0]

    cores = []
    for c in range(N_CORES):
        lo, hi = c * npc, min((c + 1) * npc, n_nodes)
        sel = np.nonzero((dst >= lo) & (dst < hi))[0]
        ldst = (dst[sel] - lo).astype(np.int64)
        n_real = hi - lo
        deg = np.bincount(ldst, minlength=npc)
        order = np.argsort(-deg, kind="stable")       # all npc local ids
        # edges sorted by local dst; esort[k] = global edge row
        order_e = np.argsort(ldst, kind="stable")
        esort = sel[order_e]
        starts = np.zeros(npc + 1, np.int64)
        np.cumsum(deg, out=starts[1:])
        cores.append(dict(lo=lo, n_real=n_real, deg=deg, order=order,
                          esort=esort, starts=starts))

    # canonical group schedule, shared by all cores
    ngrp = -(-npc // P)
    ngrp = -(-ngrp // SG) * SG                # round up to supergroup multiple
    npc_pad = ngrp * P
    d_list = np.ones(ngrp, np.int64)
    for c in cores:
        degs = np.zeros(npc_pad, np.int64)
        degs[:npc] = c["deg"][c["order"]]
        dg = degs.reshape(ngrp, P).max(axis=1)
        d_list = np.maximum(d_list, dg)
    d_list = np.maximum(d_list, 1)
    # supergroup slab layout: per SG one contiguous [128, W_s] block
    # (partition-major) where W_s = sum of the SG's D_g * 128 columns.
    nsg = ngrp // SG
    w_sg = np.array([int(d_list[s * SG:(s + 1) * SG].sum()) * P
                     for s in range(nsg)], np.int64)
    sg_offs = np.zeros(nsg + 1, np.int64)
    np.cumsum(w_sg * P, out=sg_offs[1:])
    tot = int(sg_offs[-1])

    in_maps = []
    for c in cores:
        slab = np.zeros(tot, np.float16)
        order = c["order"]
        deg, starts, esort = c["deg"], c["starts"], c["esort"]
        for s in range(nsg):
            block_cols = []
            for gi in range(SG):
                g = s * SG + gi
                d = int(d_list[g])
                blk = np.zeros((P, d * P), np.float16)
                nid = order[g * P:(g + 1) * P]         # may be short at tail
                if len(nid):
                    degs_g = deg[nid][:, None]
                    ks = np.arange(d)[None, :]
                    valid = ks < degs_g
                    pos = starts[nid][:, None] + ks
                    rows = np.where(valid,
                                    esort[np.minimum(pos, len(esort) - 1)],
                                    zero_row)
                    blk[:len(nid)] = e16z[rows].reshape(len(nid), d * P)
                block_cols.append(blk)
            sg_block = np.concatenate(block_cols, axis=1)  # [128, W_s]
            slab[sg_offs[s]:sg_offs[s + 1]] = sg_block.reshape(-1)
        in_maps.append(dict(e_ell=slab))

    meta = dict(npc=npc, ngrp=ngrp, npc_pad=npc_pad,
                d_list=d_list, sg_offs=sg_offs, w_sg=w_sg, tot=tot,
                cores=cores)
    return in_maps, meta


def _host_prep_x(x, meta):
    """Build per-core x buffers (node-major f32 + feature-major fp16)."""
    npc, npc_pad = meta["npc"], meta["npc_pad"]
    out = []
    for c in meta["cores"]:
        xp = np.zeros((npc_pad, P), np.float32)
        xr = np.asarray(x[c["lo"]:c["lo"] + c["n_real"]], np.float32)
        perm = c["order"]
        # rows beyond n_real in `perm` index nodes that don't exist for the
        # tail core; keep them zero.
        valid = perm < c["n_real"]
        xp[np.nonzero(valid)[0]] = xr[perm[valid]]
        xt = np.ascontiguousarray(xp.T.astype(np.float16))
        out.append((xp, xt))
    return out


# --------------------------------------------------------------------------
# Device program
# --------------------------------------------------------------------------

def _build_program(meta, flags):
    ngrp, npc_pad = meta["ngrp"], meta["npc_pad"]
    d_list = meta["d_list"]
    sg_offs, w_sg, tot = meta["sg_offs"], meta["w_sg"], meta["tot"]
    use_bo = flags["use_bo"]
    use_gn = flags["use_gn"]

    nc = bacc.Bacc("TRN2", target_bir_lowering=False, debug=False)

    e_ell = nc.dram_tensor("e_ell", [tot], F16, kind="ExternalInput").ap()
    xT_d = nc.dram_tensor("xT", [P, npc_pad], F16, kind="ExternalInput").ap()
    x_d = nc.dram_tensor("x_nm", [npc_pad, P], F32, kind="ExternalInput").ap()
    w0x_d = nc.dram_tensor("W0x", [P, P], F16, kind="ExternalInput").ap()
    w0m_d = nc.dram_tensor("W0m", [P, P], F16, kind="ExternalInput").ap()
    wh0_d = nc.dram_tensor("Wh0", [P, P], F16, kind="ExternalInput").ap()
    wh1_d = nc.dram_tensor("Wh1", [P, P], F16, kind="ExternalInput").ap()
    wo_d = nc.dram_tensor("Wo", [P, P], F16, kind="ExternalInput").ap()
    b0_d = nc.dram_tensor("b0", [P, 1], F32, kind="ExternalInput").ap()
    bh0_d = nc.dram_tensor("bh0", [P, 1], F32, kind="ExternalInput").ap()
    bh1_d = nc.dram_tensor("bh1", [P, 1], F32, kind="ExternalInput").ap()
    i_d = nc.dram_tensor("I16", [P, P], F16, kind="ExternalInput").ap()
    if use_bo:
        bo_d = nc.dram_tensor("bo_b", [P, SG * P], F32,
                              kind="ExternalInput").ap()
    if use_gn:
        gnw_d = nc.dram_tensor("gnw_b", [P, SG * P], F32,
                               kind="ExternalInput").ap()
        gnb_d = nc.dram_tensor("gnb_b", [P, SG * P], F32,
                               kind="ExternalInput").ap()
    out_d = nc.dram_tensor("out", [npc_pad, P], F32, kind="ExternalOutput").ap()

    nsg = ngrp // SG
    W = SG * P

    with tile.TileContext(nc) as tc:
        with (
            tc.tile_pool(name="const", bufs=1) as cpool,
            tc.tile_pool(name="slab", bufs=3) as spool,
            tc.tile_pool(name="io", bufs=3) as iopool,
            tc.tile_pool(name="act", bufs=2) as apool,
            tc.tile_pool(name="gn", bufs=2) as gpool,
            tc.tile_pool(name="stat", bufs=2) as tpool,
            tc.tile_pool(name="pmsg", bufs=2, space="PSUM") as pmsg,
            tc.tile_pool(name="pmlp", bufs=3, space="PSUM") as pmlp,
            tc.tile_pool(name="pout", bufs=2, space="PSUM") as pout,
        ):
            ident = cpool.tile([P, P], F16)
            nc.sync.dma_start(ident[:], i_d[:])
            eps_t = cpool.tile([P, 1], F32)
            nc.vector.memset(eps_t[:], EPS)
            w0x = cpool.tile([P, P], F16)
            nc.sync.dma_start(w0x[:], w0x_d[:])
            w0m = cpool.tile([P, P], F16)
            nc.sync.dma_start(w0m[:], w0m_d[:])
            wh0 = cpool.tile([P, P], F16)
            nc.sync.dma_start(wh0[:], wh0_d[:])
            wh1 = cpool.tile([P, P], F16)
            nc.sync.dma_start(wh1[:], wh1_d[:])
            wo = cpool.tile([P, P], F16)
            nc.sync.dma_start(wo[:], wo_d[:])
            b0 = cpool.tile([P, 1], F32)
            nc.sync.dma_start(b0[:], b0_d[:])
            bh0 = cpool.tile([P, 1], F32)
            nc.sync.dma_start(bh0[:], bh0_d[:])
            bh1 = cpool.tile([P, 1], F32)
            nc.sync.dma_start(bh1[:], bh1_d[:])
            if use_bo:
                bo_b = cpool.tile([P, W], F32)
                nc.sync.dma_start(bo_b[:], bo_d[:])
            if use_gn:
                gnw_b = cpool.tile([P, W], F32)
                nc.sync.dma_start(gnw_b[:], gnw_d[:])
                gnb_b = cpool.tile([P, W], F32)
                nc.sync.dma_start(gnb_b[:], gnb_d[:])

            for s in range(nsg):
                g0 = s * SG
                ws = int(w_sg[s])
                slab = spool.tile([P, ws], F16, tag="slab")
                nc.sync.dma_start(
                    slab[:],
                    e_ell[int(sg_offs[s]):int(sg_offs[s + 1])]
                    .rearrange("(p w) -> p w", p=P),
                )
                # ---- message scatter: msgT[feat, node] ----
                msg_ps = pmsg.tile([P, W], F32, tag="msg")
                col = 0
                for gi in range(SG):
                    d = int(d_list[g0 + gi])
                    dst = msg_ps[:, gi * P:(gi + 1) * P]
                    for k in range(d):
                        nc.tensor.matmul(
                            dst,
                            lhsT=slab[:, (col + k) * P:(col + k + 1) * P],
                            rhs=ident[:],
                            start=(k == 0),
                            stop=(k == d - 1),
                        )
                    col += d
                msg_s = apool.tile([P, W], F16, tag="msg_s")
                nc.scalar.copy(msg_s[:], msg_ps[:])

                # ---- MLP (feature-major, fp16 in / f32 accum) ----
                xt = iopool.tile([P, W], F16, tag="xT")
                nc.sync.dma_start(xt[:], xT_d[:, s * W:(s + 1) * W])

                h_ps = pmlp.tile([P, W], F32, tag="mlp")
                nc.tensor.matmul(h_ps[:], lhsT=w0x[:], rhs=xt[:],
                                 start=True, stop=False)
                nc.tensor.matmul(h_ps[:], lhsT=w0m[:], rhs=msg_s[:],
                                 start=False, stop=True)
                h1 = apool.tile([P, W], F16, tag="h")
                nc.scalar.activation(h1[:], h_ps[:], AF.Relu, bias=b0[:, 0:1])

                h_ps2 = pmlp.tile([P, W], F32, tag="mlp")
                nc.tensor.matmul(h_ps2[:], lhsT=wh0[:], rhs=h1[:],
                                 start=True, stop=True)
                h2 = apool.tile([P, W], F16, tag="h")
                nc.scalar.activation(h2[:], h_ps2[:], AF.Relu, bias=bh0[:, 0:1])

                h_ps3 = pmlp.tile([P, W], F32, tag="mlp")
                nc.tensor.matmul(h_ps3[:], lhsT=wh1[:], rhs=h2[:],
                                 start=True, stop=True)
                h3 = apool.tile([P, W], F16, tag="h")
                nc.scalar.activation(h3[:], h_ps3[:], AF.Relu, bias=bh1[:, 0:1])

                # ---- output layer, node-major out[node, ch] ----
                o_ps = pout.tile([P, W], F32, tag="o")
                for gi in range(SG):
                    nc.tensor.matmul(
                        o_ps[:, gi * P:(gi + 1) * P],
                        lhsT=h3[:, gi * P:(gi + 1) * P],
                        rhs=wo[:],
                        start=True, stop=True,
                    )

                # ---- GroupNorm(1, C) + residual ----
                if use_bo:
                    basis = gpool.tile([P, W], F32, tag="basis")
                    nc.vector.tensor_add(basis[:], o_ps[:], bo_b[:])
                else:
                    basis = o_ps
                b3 = basis[:].rearrange("p (g c) -> p g c", c=P)
                s1 = tpool.tile([P, SG], F32, tag="s1")
                nc.vector.tensor_reduce(s1[:], b3, axis=mybir.AxisListType.X,
                                        op=ALU.add)
                s2 = tpool.tile([P, SG], F32, tag="s2")
                sq = gpool.tile([P, P], F32, tag="sq")
                for gi in range(SG):
                    nc.scalar.activation(
                        sq[:], basis[:, gi * P:(gi + 1) * P], AF.Square,
                        accum_out=s2[:, gi:gi + 1])
                mu = tpool.tile([P, SG], F32, tag="mu")
                nc.vector.tensor_scalar_mul(mu[:], s1[:], 1.0 / P)
                ex2 = tpool.tile([P, SG], F32, tag="ex2")
                nc.vector.tensor_scalar_mul(ex2[:], s2[:], 1.0 / P)
                var = tpool.tile([P, SG], F32, tag="var")
                nc.vector.tensor_tensor(var[:], mu[:], mu[:], op=ALU.mult)
                nc.vector.tensor_tensor(var[:], ex2[:], var[:],
                                        op=ALU.subtract)
                sd = tpool.tile([P, SG], F32, tag="sd")
                nc.scalar.activation(sd[:], var[:], AF.Sqrt,
                                     bias=eps_t[:, 0:1])
                rinv = tpool.tile([P, SG], F32, tag="rinv")
                nc.vector.reciprocal(rinv[:], sd[:])
                negms = tpool.tile([P, SG], F32, tag="negms")
                nc.vector.tensor_tensor(negms[:], mu[:], rinv[:], op=ALU.mult)
                nc.vector.tensor_scalar_mul(negms[:], negms[:], -1.0)

                xres = iopool.tile([P, W], F32, tag="xres")
                nc.sync.dma_start(
                    xres[:].rearrange("p (g f) -> p g f", f=P),
                    x_d[g0 * P:(g0 + SG) * P, :]
                    .rearrange("(g p) f -> p g f", p=P))
                ot = gpool.tile([P, W], F32, tag="ot")
                for gi in range(SG):
                    u = gpool.tile([P, P], F32, tag="u")
                    nc.vector.tensor_scalar(
                        u[:], basis[:, gi * P:(gi + 1) * P],
                        scalar1=rinv[:, gi:gi + 1],
                        scalar2=negms[:, gi:gi + 1],
                        op0=ALU.mult, op1=ALU.add)
                    if use_gn:
                        nc.vector.tensor_tensor(
                            u[:], u[:], gnw_b[:, gi * P:(gi + 1) * P],
                            op=ALU.mult)
                        nc.vector.tensor_tensor(
                            u[:], u[:], gnb_b[:, gi * P:(gi + 1) * P],
                            op=ALU.add)
                    nc.vector.tensor_add(ot[:, gi * P:(gi + 1) * P], u[:],
                                         xres[:, gi * P:(gi + 1) * P])
                nc.sync.dma_start(
                    out_d[g0 * P:(g0 + SG) * P, :]
                    .rearrange("(g p) f -> p g f", p=P),
                    ot[:].rearrange("p (g f) -> p g f", f=P))

    return nc


# --------------------------------------------------------------------------
# Entry point
# --------------------------------------------------------------------------

def _run(inputs, trace=False):
    x = np.asarray(inputs["x"], np.float32)
    e = np.asarray(inputs["e"], np.float32)
    edge_index = np.asarray(inputs["edge_index"])
    W0 = np.asarray(inputs["W0"], np.float32)
    b0 = np.asarray(inputs["b0"], np.float32)
    Wh = np.asarray(inputs["Wh"], np.float32)
    bh = np.asarray(inputs["bh"], np.float32)
    Wo = np.asarray(inputs["Wo"], np.float32)
    bo = np.asarray(inputs["bo"], np.float32)
    gn_w = np.asarray(inputs["gn_w"], np.float32)
    gn_b = np.asarray(inputs["gn_b"], np.float32)

    import time as _time
    _t0 = _time.monotonic()
    in_maps, meta = _host_prep(x, e, edge_index)
    xbufs = _host_prep_x(x, meta)
    print(f"[kernel] host prep {_time.monotonic()-_t0:.1f}s", flush=True)

    flags = dict(use_bo=bool(np.any(bo != 0.0)),
                 use_gn=bool(np.any(gn_w != 1.0) or np.any(gn_b != 0.0)))

    consts = dict(
        W0x=np.ascontiguousarray(W0[:P], np.float16),
        W0m=np.ascontiguousarray(W0[P:], np.float16),
        Wh0=np.ascontiguousarray(Wh[0], np.float16),
        Wh1=np.ascontiguousarray(Wh[1], np.float16),
        Wo=np.ascontiguousarray(Wo, np.float16),
        b0=b0.reshape(P, 1).copy(),
        bh0=bh[0].reshape(P, 1).copy(),
        bh1=bh[1].reshape(P, 1).copy(),
        I16=np.eye(P, dtype=np.float16),
    )
    if flags["use_bo"]:
        consts["bo_b"] = np.tile(bo[None, :], (P, SG)).astype(np.float32)
    if flags["use_gn"]:
        consts["gnw_b"] = np.tile(gn_w[None, :], (P, SG)).astype(np.float32)
        consts["gnb_b"] = np.tile(gn_b[None, :], (P, SG)).astype(np.float32)

    _t0 = _time.monotonic()
    nc = _build_program(meta, flags)
    print(f"[kernel] build {_time.monotonic()-_t0:.1f}s", flush=True)
    _t0 = _time.monotonic()
    nc.compile()
    print(f"[kernel] bacc compile {_time.monotonic()-_t0:.1f}s", flush=True)
    _t0 = _time.monotonic()

    full_maps = []
    for c in range(N_CORES):
        m = dict(in_maps[c])
        m["x_nm"], m["xT"] = xbufs[c]
        m.update(consts)
        full_maps.append(m)

    res = run_bass_kernel_spmd(nc, full_maps, list(range(N_CORES)),
                               trace=trace)
    print(f"[kernel] spmd run {_time.monotonic()-_t0:.1f}s", flush=True)

    n_nodes = x.shape[0]
    out = np.empty((n_nodes, P), np.float32)
    npc = meta["npc"]
    for c, cinfo in enumerate(meta["cores"]):
        oc = res.results[c]["out"]
        perm = cinfo["order"]
        valid = perm < cinfo["n_real"]
        rows = np.nonzero(valid)[0]
        out[cinfo["lo"] + perm[valid]] = oc[rows]
    return out, res


def kernel(**inputs):
    out, _ = _run(inputs, trace=False)
    return out


# revision 29
# speedup vs baseline: 1.4087x; 1.0671x over previous
"""Trainium2 Bass kernel for nn_NodeConv (GNN message passing).

Strategy (8 NeuronCores, data-parallel, no collectives):
  - Nodes are partitioned into 8 contiguous ranges; every edge is routed to
    the core that owns its *destination* node, so the segment-sum is fully
    local to each core.  MLP weights are replicated.
  - On the host, each core's nodes are sorted by in-degree and packed into
    groups of 128.  Edge features are laid out in an ELL-style slab
    [128 nodes x D_g chunks x 128 feat] (D_g = max degree in group, ~1-2%
    padding thanks to the degree sort).
  - On the device the segment-sum is performed by the TensorEngine:
    for each chunk, matmul(lhsT=chunk, rhs=I128) accumulates chunk^T into
    PSUM, yielding the per-group message matrix *feature-major* with zero
    per-edge elementwise work.  Edge features travel as fp16 (~5e-4 rel err).
  - The MLP runs feature-major (weights as lhsT), the last layer uses the
    activations as lhsT which transposes the result back to node-major for
    the GroupNorm + residual, and the output is DMA'd node-major.
"""

import sys

sys.path.insert(0, "/opt/trn_rl_repo")

import numpy as np

import concourse.bass as bass
import concourse.bacc as bacc
import concourse.tile as tile
from concourse import mybir
from concourse.bass_utils import run_bass_kernel_spmd

# bass_utils imports antenv.axon_hooks unconditionally when tracing is
# requested; the image's antenv lacks that module.  Provide a null registry
# so a BASS_TRACE env var can't crash the run.
try:
    import antenv.axon_hooks  # noqa: F401
except ImportError:
    import types as _types
    import antenv as _antenv
    _m = _types.ModuleType("antenv.axon_hooks")
    _m._hook = None
    _m.set_axon_ntff_profile_hook = lambda h, _m=_m: setattr(_m, "_hook", h)
    _m.get_axon_ntff_profile_hook = lambda _m=_m: _m._hook
    sys.modules["antenv.axon_hooks"] = _m
    _antenv.axon_hooks = _m

P = 128
N_CORES = 8
SG = 4          # groups per supergroup (MLP batch = 512 nodes)
EPS = 1e-5

F16 = mybir.dt.float16
F32 = mybir.dt.float32
AF = mybir.ActivationFunctionType
# tensor_tensor_reduce faults on HW (passes CoreSim/walrus) — keep it off.
USE_ACT_DMA = True
USE_TTR = False
USE_ACT_U = True
ALU = mybir.AluOpType


# --------------------------------------------------------------------------
# Host-side sharding / layout
# --------------------------------------------------------------------------

def _host_prep(x, e, edge_index):
    """Shard nodes/edges across cores and build per-core ELL slabs."""
    n_nodes = x.shape[0]
    npc = -(-n_nodes // N_CORES)              # nodes per core (ceil)
    dst = np.asarray(edge_index[1]).astype(np.int64)
    e16 = np.ascontiguousarray(e, dtype=np.float16)
    e16z = np.vstack([e16, np.zeros((1, e16.shape[1]), np.float16)])
    zero_row = e16.shape[0]

    cores = []
    for c in range(N_CORES):
        lo, hi = c * npc, min((c + 1) * npc, n_nodes)
        sel = np.nonzero((dst >= lo) & (dst < hi))[0]
        ldst = (dst[sel] - lo).astype(np.int64)
        n_real = hi - lo
        deg = np.bincount(ldst, minlength=npc)
        order = np.argsort(-deg, kind="stable")       # all npc local ids
        # edges sorted by local dst; esort[k] = global edge row
        order_e = np.argsort(ldst, kind="stable")
        esort = sel[order_e]
        starts = np.zeros(npc + 1, np.int64)
        np.cumsum(deg, out=starts[1:])
        cores.append(dict(lo=lo, n_real=n_real, deg=deg, order=order,
                          esort=esort, starts=starts))

    # canonical group schedule, shared by all cores
    ngrp = -(-npc // P)
    ngrp = -(-ngrp // SG) * SG                # round up to supergroup multiple
    npc_pad = ngrp * P
    d_list = np.ones(ngrp, np.int64)
    for c in cores:
        degs = np.zeros(npc_pad, np.int64)
        degs[:npc] = c["deg"][c["order"]]
        dg = degs.reshape(ngrp, P).max(axis=1)
        d_list = np.maximum(d_list, dg)
    d_list = np.maximum(d_list, 1)
    # supergroup slab layout: per SG one contiguous [128, W_s] block
    # (partition-major) where W_s = sum of the SG's D_g * 128 columns.
    nsg = ngrp // SG
    w_sg = np.array([int(d_list[s * SG:(s + 1) * SG].sum()) * P
                     for s in range(nsg)], np.int64)
    sg_offs = np.zeros(nsg + 1, np.int64)
    np.cumsum(w_sg * P, out=sg_offs[1:])
    tot = int(sg_offs[-1])

    in_maps = []
    for c in cores:
        slab = np.zeros(tot, np.float16)
        order = c["order"]
        deg, starts, esort = c["deg"], c["starts"], c["esort"]
        for s in range(nsg):
            block_cols = []
            for gi in range(SG):
                g = s * SG + gi
                d = int(d_list[g])
                blk = np.zeros((P, d * P), np.float16)
                nid = order[g * P:(g + 1) * P]         # may be short at tail
                if len(nid):
                    degs_g = deg[nid][:, None]
                    ks = np.arange(d)[None, :]
                    valid = ks < degs_g
                    pos = starts[nid][:, None] + ks
                    rows = np.where(valid,
                                    esort[np.minimum(pos, len(esort) - 1)],
                                    zero_row)
                    blk[:len(nid)] = e16z[rows].reshape(len(nid), d * P)
                block_cols.append(blk)
            sg_block = np.concatenate(block_cols, axis=1)  # [128, W_s]
            slab[sg_offs[s]:sg_offs[s + 1]] = sg_block.reshape(-1)
        in_maps.append(dict(e_ell=slab))

    meta = dict(npc=npc, ngrp=ngrp, npc_pad=npc_pad,
                d_list=d_list, sg_offs=sg_offs, w_sg=w_sg, tot=tot,
                cores=cores)
    return in_maps, meta


def _host_prep_x(x, meta):
    """Per-core x buffers: partition-major f32 (for the residual; row p holds
    nodes p, 128+p, 256+p, ... interleaved by group) + feature-major fp16."""
    npc, npc_pad, ngrp = meta["npc"], meta["npc_pad"], meta["ngrp"]
    out = []
    for c in meta["cores"]:
        xp = np.zeros((npc_pad, P), np.float32)
        xr = np.asarray(x[c["lo"]:c["lo"] + c["n_real"]], np.float32)
        perm = c["order"]
        # rows beyond n_real in `perm` index nodes that don't exist for the
        # tail core; keep them zero.
        valid = perm < c["n_real"]
        xp[np.nonzero(valid)[0]] = xr[perm[valid]]
        # [grp, p, f] -> [p, grp*f]: per-partition contiguous per group
        xpm = np.ascontiguousarray(
            xp.reshape(ngrp, P, P).transpose(1, 0, 2).reshape(P, ngrp * P))
        xt = np.ascontiguousarray(xp.T.astype(np.float16))
        out.append((xpm, xt))
    return out


# --------------------------------------------------------------------------
# Device program
# --------------------------------------------------------------------------

def _build_program(meta, flags):
    ngrp, npc_pad = meta["ngrp"], meta["npc_pad"]
    d_list = meta["d_list"]
    sg_offs, w_sg, tot = meta["sg_offs"], meta["w_sg"], meta["tot"]
    use_bo = flags["use_bo"]
    use_gn = flags["use_gn"]

    nc = bacc.Bacc("TRN2", target_bir_lowering=False, debug=False)

    e_ell = nc.dram_tensor("e_ell", [tot], F16, kind="ExternalInput").ap()
    xT_d = nc.dram_tensor("xT", [P, npc_pad], F16, kind="ExternalInput").ap()
    x_d = nc.dram_tensor("x_pm", [P, npc_pad], F32, kind="ExternalInput").ap()
    w0x_d = nc.dram_tensor("W0x", [P, P], F16, kind="ExternalInput").ap()
    w0m_d = nc.dram_tensor("W0m", [P, P], F16, kind="ExternalInput").ap()
    wh0_d = nc.dram_tensor("Wh0", [P, P], F16, kind="ExternalInput").ap()
    wh1_d = nc.dram_tensor("Wh1", [P, P], F16, kind="ExternalInput").ap()
    wo_d = nc.dram_tensor("Wo", [P, P], F16, kind="ExternalInput").ap()
    b0_d = nc.dram_tensor("b0", [P, 1], F32, kind="ExternalInput").ap()
    bh0_d = nc.dram_tensor("bh0", [P, 1], F32, kind="ExternalInput").ap()
    bh1_d = nc.dram_tensor("bh1", [P, 1], F32, kind="ExternalInput").ap()
    i_d = nc.dram_tensor("I16", [P, P], F16, kind="ExternalInput").ap()
    if use_bo:
        bo_d = nc.dram_tensor("bo_b", [P, SG * P], F32,
                              kind="ExternalInput").ap()
    if use_gn:
        gnw_d = nc.dram_tensor("gnw_b", [P, SG * P], F32,
                               kind="ExternalInput").ap()
        gnb_d = nc.dram_tensor("gnb_b", [P, SG * P], F32,
                               kind="ExternalInput").ap()
    out_d = nc.dram_tensor("out", [P, npc_pad], F32, kind="ExternalOutput").ap()

    nsg = ngrp // SG
    W = SG * P

    with tile.TileContext(nc) as tc:
        with (
            tc.tile_pool(name="const", bufs=1) as cpool,
            tc.tile_pool(name="slab", bufs=3) as spool,
            tc.tile_pool(name="io", bufs=3) as iopool,
            tc.tile_pool(name="act", bufs=2) as apool,
            tc.tile_pool(name="gn", bufs=2) as gpool,
            tc.tile_pool(name="stat", bufs=2) as tpool,
            tc.tile_pool(name="pmsg", bufs=2, space="PSUM") as pmsg,
            tc.tile_pool(name="pmlp", bufs=3, space="PSUM") as pmlp,
            tc.tile_pool(name="pout", bufs=2, space="PSUM") as pout,
        ):
            ident = cpool.tile([P, P], F16)
            nc.sync.dma_start(ident[:], i_d[:])
            eps_t = cpool.tile([P, 1], F32)
            nc.vector.memset(eps_t[:], EPS)
            w0x = cpool.tile([P, P], F16)
            nc.sync.dma_start(w0x[:], w0x_d[:])
            w0m = cpool.tile([P, P], F16)
            nc.sync.dma_start(w0m[:], w0m_d[:])
            wh0 = cpool.tile([P, P], F16)
            nc.sync.dma_start(wh0[:], wh0_d[:])
            wh1 = cpool.tile([P, P], F16)
            nc.sync.dma_start(wh1[:], wh1_d[:])
            wo = cpool.tile([P, P], F16)
            nc.sync.dma_start(wo[:], wo_d[:])
            b0 = cpool.tile([P, 1], F32)
            nc.sync.dma_start(b0[:], b0_d[:])
            bh0 = cpool.tile([P, 1], F32)
            nc.sync.dma_start(bh0[:], bh0_d[:])
            bh1 = cpool.tile([P, 1], F32)
            nc.sync.dma_start(bh1[:], bh1_d[:])
            if use_bo:
                bo_b = cpool.tile([P, W], F32)
                nc.sync.dma_start(bo_b[:], bo_d[:])
            if use_gn:
                gnw_b = cpool.tile([P, W], F32)
                nc.sync.dma_start(gnw_b[:], gnw_d[:])
                gnb_b = cpool.tile([P, W], F32)
                nc.sync.dma_start(gnb_b[:], gnb_d[:])

            for s in range(nsg):
                g0 = s * SG
                ws = int(w_sg[s])
                slab = spool.tile([P, ws], F16, tag="slab")
                nc.sync.dma_start(
                    slab[:],
                    e_ell[int(sg_offs[s]):int(sg_offs[s + 1])]
                    .rearrange("(p w) -> p w", p=P),
                )
                # ---- message scatter: msgT[feat, node] ----
                msg_ps = pmsg.tile([P, W], F32, tag="msg")
                col = 0
                for gi in range(SG):
                    d = int(d_list[g0 + gi])
                    dst = msg_ps[:, gi * P:(gi + 1) * P]
                    for k in range(d):
                        nc.tensor.matmul(
                            dst,
                            lhsT=slab[:, (col + k) * P:(col + k + 1) * P],
                            rhs=ident[:],
                            start=(k == 0),
                            stop=(k == d - 1),
                        )
                    col += d
                msg_s = apool.tile([P, W], F16, tag="msg_s")
                nc.scalar.copy(msg_s[:], msg_ps[:])

                # ---- MLP (feature-major, fp16 in / f32 accum) ----
                xt = iopool.tile([P, W], F16, tag="xT")
                dma_eng = nc.scalar if USE_ACT_DMA else nc.sync
                dma_eng.dma_start(xt[:], xT_d[:, s * W:(s + 1) * W])

                h_ps = pmlp.tile([P, W], F32, tag="mlp")
                nc.tensor.matmul(h_ps[:], lhsT=w0x[:], rhs=xt[:],
                                 start=True, stop=False)
                nc.tensor.matmul(h_ps[:], lhsT=w0m[:], rhs=msg_s[:],
                                 start=False, stop=True)
                h1 = apool.tile([P, W], F16, tag="h")
                nc.scalar.activation(h1[:], h_ps[:], AF.Relu, bias=b0[:, 0:1])

                h_ps2 = pmlp.tile([P, W], F32, tag="mlp")
                nc.tensor.matmul(h_ps2[:], lhsT=wh0[:], rhs=h1[:],
                                 start=True, stop=True)
                h2 = apool.tile([P, W], F16, tag="h")
                nc.scalar.activation(h2[:], h_ps2[:], AF.Relu, bias=bh0[:, 0:1])

                h_ps3 = pmlp.tile([P, W], F32, tag="mlp")
                nc.tensor.matmul(h_ps3[:], lhsT=wh1[:], rhs=h2[:],
                                 start=True, stop=True)
                h3 = apool.tile([P, W], F16, tag="h")
                nc.scalar.activation(h3[:], h_ps3[:], AF.Relu, bias=bh1[:, 0:1])

                # ---- output layer, node-major out[node, ch] ----
                o_ps = pout.tile([P, W], F32, tag="o")
                for gi in range(SG):
                    nc.tensor.matmul(
                        o_ps[:, gi * P:(gi + 1) * P],
                        lhsT=h3[:, gi * P:(gi + 1) * P],
                        rhs=wo[:],
                        start=True, stop=True,
                    )

                # ---- GroupNorm(1, C) + residual ----
                basis = gpool.tile([P, W], F32, tag="basis")
                if use_bo:
                    nc.vector.tensor_add(basis[:], o_ps[:], bo_b[:])
                else:
                    nc.vector.tensor_copy(basis[:], o_ps[:])
                b3 = basis[:].rearrange("p (g c) -> p g c", c=P)
                s1 = tpool.tile([P, SG], F32, tag="s1")
                nc.vector.tensor_reduce(s1[:], b3, axis=mybir.AxisListType.X,
                                        op=ALU.add)
                s2 = tpool.tile([P, SG], F32, tag="s2")
                sq = gpool.tile([P, P], F32, tag="sq")
                if USE_TTR:
                    for gi in range(SG):
                        bs = basis[:, gi * P:(gi + 1) * P]
                        nc.vector.tensor_tensor_reduce(
                            sq[:], bs, bs, scale=1.0, scalar=0.0,
                            op0=ALU.mult, op1=ALU.add,
                            accum_out=s2[:, gi:gi + 1])
                else:
                    sqf = gpool.tile([P, W], F32, tag="sqf")
                    nc.scalar.square(sqf[:], basis[:])
                    nc.vector.tensor_reduce(
                        s2[:], sqf[:].rearrange("p (g c) -> p g c", c=P),
                        axis=mybir.AxisListType.X, op=ALU.add)
                mu = tpool.tile([P, SG], F32, tag="mu")
                nc.vector.tensor_scalar_mul(mu[:], s1[:], 1.0 / P)
                ex2 = tpool.tile([P, SG], F32, tag="ex2")
                nc.vector.tensor_scalar_mul(ex2[:], s2[:], 1.0 / P)
                var = tpool.tile([P, SG], F32, tag="var")
                nc.vector.tensor_tensor(var[:], mu[:], mu[:], op=ALU.mult)
                nc.vector.tensor_tensor(var[:], ex2[:], var[:],
                                        op=ALU.subtract)
                sd = tpool.tile([P, SG], F32, tag="sd")
                nc.scalar.activation(sd[:], var[:], AF.Sqrt,
                                     bias=eps_t[:, 0:1])
                rinv = tpool.tile([P, SG], F32, tag="rinv")
                nc.vector.reciprocal(rinv[:], sd[:])
                negms = tpool.tile([P, SG], F32, tag="negms")
                nc.vector.tensor_tensor(negms[:], mu[:], rinv[:], op=ALU.mult)
                nc.vector.tensor_scalar_mul(negms[:], negms[:], -1.0)

                xres = iopool.tile([P, W], F32, tag="xres")
                dma_eng.dma_start(xres[:], x_d[:, s * W:(s + 1) * W])
                ot = gpool.tile([P, W], F32, tag="ot")
                for gi in range(SG):
                    u = gpool.tile([P, P], F32, tag="u")
                    if USE_ACT_U:
                        nc.scalar.activation(
                            u[:], basis[:, gi * P:(gi + 1) * P], AF.Identity,
                            bias=negms[:, gi:gi + 1], scale=rinv[:, gi:gi + 1])
                    else:
                        nc.vector.tensor_scalar(
                            u[:], basis[:, gi * P:(gi + 1) * P],
                            scalar1=rinv[:, gi:gi + 1],
                            scalar2=negms[:, gi:gi + 1],
                            op0=ALU.mult, op1=ALU.add)
                    if use_gn:
                        nc.vector.tensor_tensor(
                            u[:], u[:], gnw_b[:, gi * P:(gi + 1) * P],
                            op=ALU.mult)
                        nc.vector.tensor_tensor(
                            u[:], u[:], gnb_b[:, gi * P:(gi + 1) * P],
                            op=ALU.add)
                    nc.vector.tensor_add(ot[:, gi * P:(gi + 1) * P], u[:],
                                         xres[:, gi * P:(gi + 1) * P])
                dma_eng.dma_start(out_d[:, s * W:(s + 1) * W], ot[:])

    return nc


# --------------------------------------------------------------------------
# Entry point
# --------------------------------------------------------------------------

def _run(inputs, trace=False):
    x = np.asarray(inputs["x"], np.float32)
    e = np.asarray(inputs["e"], np.float32)
    edge_index = np.asarray(inputs["edge_index"])
    W0 = np.asarray(inputs["W0"], np.float32)
    b0 = np.asarray(inputs["b0"], np.float32)
    Wh = np.asarray(inputs["Wh"], np.float32)
    bh = np.asarray(inputs["bh"], np.float32)
    Wo = np.asarray(inputs["Wo"], np.float32)
    bo = np.asarray(inputs["bo"], np.float32)
    gn_w = np.asarray(inputs["gn_w"], np.float32)
    gn_b = np.asarray(inputs["gn_b"], np.float32)

    import time as _time
    _t0 = _time.monotonic()
    in_maps, meta = _host_prep(x, e, edge_index)
    xbufs = _host_prep_x(x, meta)
    print(f"[kernel] host prep {_time.monotonic()-_t0:.1f}s", flush=True)

    flags = dict(use_bo=bool(np.any(bo != 0.0)),
                 use_gn=bool(np.any(gn_w != 1.0) or np.any(gn_b != 0.0)))

    consts = dict(
        W0x=np.ascontiguousarray(W0[:P], np.float16),
        W0m=np.ascontiguousarray(W0[P:], np.float16),
        Wh0=np.ascontiguousarray(Wh[0], np.float16),
        Wh1=np.ascontiguousarray(Wh[1], np.float16),
        Wo=np.ascontiguousarray(Wo, np.float16),
        b0=b0.reshape(P, 1).copy(),
        bh0=bh[0].reshape(P, 1).copy(),
        bh1=bh[1].reshape(P, 1).copy(),
        I16=np.eye(P, dtype=np.float16),
    )
    if flags["use_bo"]:
        consts["bo_b"] = np.tile(bo[None, :], (P, SG)).astype(np.float32)
    if flags["use_gn"]:
        consts["gnw_b"] = np.tile(gn_w[None, :], (P, SG)).astype(np.float32)
        consts["gnb_b"] = np.tile(gn_b[None, :], (P, SG)).astype(np.float32)

    _t0 = _time.monotonic()
    nc = _build_program(meta, flags)
    print(f"[kernel] build {_time.monotonic()-_t0:.1f}s", flush=True)
    _t0 = _time.monotonic()
    nc.compile()
    print(f"[kernel] bacc compile {_time.monotonic()-_t0:.1f}s", flush=True)
    _t0 = _time.monotonic()

    full_maps = []
    for c in range(N_CORES):
        m = dict(in_maps[c])
        m["x_pm"], m["xT"] = xbufs[c]
        m.update(consts)
        full_maps.append(m)

    res = run_bass_kernel_spmd(nc, full_maps, list(range(N_CORES)),
                               trace=trace)
    print(f"[kernel] spmd run {_time.monotonic()-_t0:.1f}s", flush=True)

    n_nodes = x.shape[0]
    out = np.empty((n_nodes, P), np.float32)
    ngrp = meta["ngrp"]
    for c, cinfo in enumerate(meta["cores"]):
        oc = np.asarray(res.results[c]["out"])          # [P, ngrp*P]
        on = oc.reshape(P, ngrp, P).transpose(1, 0, 2).reshape(ngrp * P, P)
        perm = cinfo["order"]
        valid = perm < cinfo["n_real"]
        rows = np.nonzero(valid)[0]
        out[cinfo["lo"] + perm[valid]] = on[rows]
    return out, res


def kernel(**inputs):
    out, _ = _run(inputs, trace=False)
    return out


# revision 33
# speedup vs baseline: 1.6231x; 1.1522x over previous
"""Trainium2 Bass kernel for nn_NodeConv (GNN message passing).

Strategy (8 NeuronCores, data-parallel, no collectives):
  - Nodes are partitioned into 8 contiguous ranges; every edge is routed to
    the core that owns its *destination* node, so the segment-sum is fully
    local to each core.  MLP weights are replicated.
  - On the host, each core's nodes are sorted by in-degree and packed into
    groups of 128.  Edge features are laid out in an ELL-style slab
    [128 nodes x D_g chunks x 128 feat] (D_g = max degree in group, ~1-2%
    padding thanks to the degree sort).
  - On the device the segment-sum is performed by the TensorEngine:
    for each chunk, matmul(lhsT=chunk, rhs=I128) accumulates chunk^T into
    PSUM, yielding the per-group message matrix *feature-major* with zero
    per-edge elementwise work.  Edge features travel as fp16 (~5e-4 rel err).
  - The MLP runs feature-major (weights as lhsT), the last layer uses the
    activations as lhsT which transposes the result back to node-major for
    the GroupNorm + residual, and the output is DMA'd node-major.
"""

import sys

sys.path.insert(0, "/opt/trn_rl_repo")

import numpy as np

import concourse.bass as bass
import concourse.bacc as bacc
import concourse.tile as tile
from concourse import mybir
from concourse.bass_utils import run_bass_kernel_spmd

# bass_utils imports antenv.axon_hooks unconditionally when tracing is
# requested; the image's antenv lacks that module.  Provide a null registry
# so a BASS_TRACE env var can't crash the run.
try:
    import antenv.axon_hooks  # noqa: F401
except ImportError:
    import types as _types
    import antenv as _antenv
    _m = _types.ModuleType("antenv.axon_hooks")
    _m._hook = None
    _m.set_axon_ntff_profile_hook = lambda h, _m=_m: setattr(_m, "_hook", h)
    _m.get_axon_ntff_profile_hook = lambda _m=_m: _m._hook
    sys.modules["antenv.axon_hooks"] = _m
    _antenv.axon_hooks = _m

P = 128
N_CORES = 8
SG = 4          # groups per supergroup (MLP batch = 512 nodes)
EPS = 1e-5

F16 = mybir.dt.float16
F32 = mybir.dt.float32
AF = mybir.ActivationFunctionType
# tensor_tensor_reduce faults on HW (passes CoreSim/walrus) — keep it off.
USE_ACT_DMA = True
USE_TTR = False
USE_ACT_U = True
ALU = mybir.AluOpType


# --------------------------------------------------------------------------
# Host-side sharding / layout
# --------------------------------------------------------------------------

def _host_prep(x, e, edge_index):
    """Shard nodes/edges across cores and build per-core ELL slabs."""
    n_nodes = x.shape[0]
    npc = -(-n_nodes // N_CORES)              # nodes per core (ceil)
    dst = np.asarray(edge_index[1]).astype(np.int64)
    e16 = np.ascontiguousarray(e, dtype=np.float16)
    e16z = np.vstack([e16, np.zeros((1, e16.shape[1]), np.float16)])
    zero_row = e16.shape[0]

    cores = []
    for c in range(N_CORES):
        lo, hi = c * npc, min((c + 1) * npc, n_nodes)
        sel = np.nonzero((dst >= lo) & (dst < hi))[0]
        ldst = (dst[sel] - lo).astype(np.int64)
        n_real = hi - lo
        deg = np.bincount(ldst, minlength=npc)
        order = np.argsort(-deg, kind="stable")       # all npc local ids
        # edges sorted by local dst; esort[k] = global edge row
        order_e = np.argsort(ldst, kind="stable")
        esort = sel[order_e]
        starts = np.zeros(npc + 1, np.int64)
        np.cumsum(deg, out=starts[1:])
        cores.append(dict(lo=lo, n_real=n_real, deg=deg, order=order,
                          esort=esort, starts=starts))

    # canonical group schedule, shared by all cores
    ngrp = -(-npc // P)
    ngrp = -(-ngrp // SG) * SG                # round up to supergroup multiple
    npc_pad = ngrp * P
    d_list = np.ones(ngrp, np.int64)
    for c in cores:
        degs = np.zeros(npc_pad, np.int64)
        degs[:npc] = c["deg"][c["order"]]
        dg = degs.reshape(ngrp, P).max(axis=1)
        d_list = np.maximum(d_list, dg)
    d_list = np.maximum(d_list, 1)
    # DMA-pair slab layout: per PAIR of supergroups one contiguous
    # [128, W_pair] partition-major block (W_pair = sum of D_g * 128 cols).
    nsg = ngrp // SG
    w_sg = np.array([int(d_list[s * SG:(s + 1) * SG].sum()) * P
                     for s in range(nsg)], np.int64)
    pairs = []                                 # (s0, n_sg)
    for s0 in range(0, nsg, 2):
        pairs.append((s0, min(2, nsg - s0)))
    w_pair = np.array([int(w_sg[s0:s0 + n].sum()) for s0, n in pairs],
                      np.int64)
    pair_offs = np.zeros(len(pairs) + 1, np.int64)
    np.cumsum(w_pair * P, out=pair_offs[1:])
    tot = int(pair_offs[-1])

    in_maps = []
    for c in cores:
        slab = np.zeros(tot, np.float16)
        order = c["order"]
        deg, starts, esort = c["deg"], c["starts"], c["esort"]
        for pi, (s0, n) in enumerate(pairs):
            block_cols = []
            for g in range(s0 * SG, (s0 + n) * SG):
                d = int(d_list[g])
                blk = np.zeros((P, d * P), np.float16)
                nid = order[g * P:(g + 1) * P]         # may be short at tail
                if len(nid):
                    degs_g = deg[nid][:, None]
                    ks = np.arange(d)[None, :]
                    valid = ks < degs_g
                    pos = starts[nid][:, None] + ks
                    rows = np.where(valid,
                                    esort[np.minimum(pos, len(esort) - 1)],
                                    zero_row)
                    blk[:len(nid)] = e16z[rows].reshape(len(nid), d * P)
                block_cols.append(blk)
            pair_block = np.concatenate(block_cols, axis=1)  # [128, W_pair]
            slab[pair_offs[pi]:pair_offs[pi + 1]] = pair_block.reshape(-1)
        in_maps.append(dict(e_ell=slab))

    meta = dict(npc=npc, ngrp=ngrp, npc_pad=npc_pad,
                d_list=d_list, pairs=pairs, pair_offs=pair_offs,
                w_pair=w_pair, w_sg=w_sg, tot=tot, cores=cores)
    return in_maps, meta


def _host_prep_x(x, meta):
    """Per-core x buffers: partition-major f32 (for the residual; row p holds
    nodes p, 128+p, 256+p, ... interleaved by group) + feature-major fp16."""
    npc, npc_pad, ngrp = meta["npc"], meta["npc_pad"], meta["ngrp"]
    out = []
    for c in meta["cores"]:
        xp = np.zeros((npc_pad, P), np.float32)
        xr = np.asarray(x[c["lo"]:c["lo"] + c["n_real"]], np.float32)
        perm = c["order"]
        # rows beyond n_real in `perm` index nodes that don't exist for the
        # tail core; keep them zero.
        valid = perm < c["n_real"]
        xp[np.nonzero(valid)[0]] = xr[perm[valid]]
        # [grp, p, f] -> [p, grp*f]: per-partition contiguous per group
        xpm = np.ascontiguousarray(
            xp.reshape(ngrp, P, P).transpose(1, 0, 2).reshape(P, ngrp * P))
        xt = np.ascontiguousarray(xp.T.astype(np.float16))
        out.append((xpm, xt))
    return out


# --------------------------------------------------------------------------
# Device program
# --------------------------------------------------------------------------

def _build_program(meta, flags):
    ngrp, npc_pad = meta["ngrp"], meta["npc_pad"]
    d_list = meta["d_list"]
    pairs, pair_offs = meta["pairs"], meta["pair_offs"]
    w_pair, tot = meta["w_pair"], meta["tot"]
    use_bo = flags["use_bo"]
    use_gn = flags["use_gn"]

    nc = bacc.Bacc("TRN2", target_bir_lowering=False, debug=False)

    e_ell = nc.dram_tensor("e_ell", [tot], F16, kind="ExternalInput").ap()
    xT_d = nc.dram_tensor("xT", [P, npc_pad], F16, kind="ExternalInput").ap()
    x_d = nc.dram_tensor("x_pm", [P, npc_pad], F32, kind="ExternalInput").ap()
    w0x_d = nc.dram_tensor("W0x", [P, P], F16, kind="ExternalInput").ap()
    w0m_d = nc.dram_tensor("W0m", [P, P], F16, kind="ExternalInput").ap()
    wh0_d = nc.dram_tensor("Wh0", [P, P], F16, kind="ExternalInput").ap()
    wh1_d = nc.dram_tensor("Wh1", [P, P], F16, kind="ExternalInput").ap()
    wo_d = nc.dram_tensor("Wo", [P, P], F16, kind="ExternalInput").ap()
    b0_d = nc.dram_tensor("b0", [P, 1], F32, kind="ExternalInput").ap()
    bh0_d = nc.dram_tensor("bh0", [P, 1], F32, kind="ExternalInput").ap()
    bh1_d = nc.dram_tensor("bh1", [P, 1], F32, kind="ExternalInput").ap()
    i_d = nc.dram_tensor("I16", [P, P], F16, kind="ExternalInput").ap()
    if use_bo:
        bo_d = nc.dram_tensor("bo_b", [P, SG * P], F32,
                              kind="ExternalInput").ap()
    if use_gn:
        gnw_d = nc.dram_tensor("gnw_b", [P, SG * P], F32,
                               kind="ExternalInput").ap()
        gnb_d = nc.dram_tensor("gnb_b", [P, SG * P], F32,
                               kind="ExternalInput").ap()
    out_d = nc.dram_tensor("out", [P, npc_pad], F32, kind="ExternalOutput").ap()

    nsg = ngrp // SG
    W = SG * P

    with tile.TileContext(nc) as tc:
        with (
            tc.tile_pool(name="const", bufs=1) as cpool,
            tc.tile_pool(name="slab", bufs=3) as spool,
            tc.tile_pool(name="io", bufs=3) as iopool,
            tc.tile_pool(name="act", bufs=2) as apool,
            tc.tile_pool(name="gn", bufs=2) as gpool,
            tc.tile_pool(name="stat", bufs=2) as tpool,
            tc.tile_pool(name="pmsg", bufs=2, space="PSUM") as pmsg,
            tc.tile_pool(name="pmlp", bufs=3, space="PSUM") as pmlp,
            tc.tile_pool(name="pout", bufs=2, space="PSUM") as pout,
        ):
            ident = cpool.tile([P, P], F16)
            nc.sync.dma_start(ident[:], i_d[:])
            eps_t = cpool.tile([P, 1], F32)
            nc.vector.memset(eps_t[:], EPS)
            w0x = cpool.tile([P, P], F16)
            nc.sync.dma_start(w0x[:], w0x_d[:])
            w0m = cpool.tile([P, P], F16)
            nc.sync.dma_start(w0m[:], w0m_d[:])
            wh0 = cpool.tile([P, P], F16)
            nc.sync.dma_start(wh0[:], wh0_d[:])
            wh1 = cpool.tile([P, P], F16)
            nc.sync.dma_start(wh1[:], wh1_d[:])
            wo = cpool.tile([P, P], F16)
            nc.sync.dma_start(wo[:], wo_d[:])
            b0 = cpool.tile([P, 1], F32)
            nc.sync.dma_start(b0[:], b0_d[:])
            bh0 = cpool.tile([P, 1], F32)
            nc.sync.dma_start(bh0[:], bh0_d[:])
            bh1 = cpool.tile([P, 1], F32)
            nc.sync.dma_start(bh1[:], bh1_d[:])
            if use_bo:
                bo_b = cpool.tile([P, W], F32)
                nc.sync.dma_start(bo_b[:], bo_d[:])
            if use_gn:
                gnw_b = cpool.tile([P, W], F32)
                nc.sync.dma_start(gnw_b[:], gnw_d[:])
                gnb_b = cpool.tile([P, W], F32)
                nc.sync.dma_start(gnb_b[:], gnb_d[:])

            dma_eng = nc.scalar if USE_ACT_DMA else nc.sync
            for pi, (s0, n_sg) in enumerate(pairs):
                wp = int(w_pair[pi])
                wn = n_sg * W
                slab = spool.tile([P, wp], F16, tag="slab")
                nc.sync.dma_start(
                    slab[:],
                    e_ell[int(pair_offs[pi]):int(pair_offs[pi + 1])]
                    .rearrange("(p w) -> p w", p=P),
                )
                xtp = iopool.tile([P, wn], F16, tag="xT")
                dma_eng.dma_start(xtp[:], xT_d[:, s0 * W:s0 * W + wn])
                xres = iopool.tile([P, wn], F32, tag="xres")
                dma_eng.dma_start(xres[:], x_d[:, s0 * W:s0 * W + wn])
                otp = gpool.tile([P, wn], F32, tag="ot")
                col = 0
                for si in range(n_sg):
                    s = s0 + si
                    g0 = s * SG
                    # ---- message scatter: msgT[feat, node] ----
                    msg_ps = pmsg.tile([P, W], F32, tag="msg")
                    for gi in range(SG):
                        d = int(d_list[g0 + gi])
                        dst = msg_ps[:, gi * P:(gi + 1) * P]
                        for k in range(d):
                            nc.tensor.matmul(
                                dst,
                                lhsT=slab[:, (col + k) * P:(col + k + 1) * P],
                                rhs=ident[:],
                                start=(k == 0),
                                stop=(k == d - 1),
                            )
                        col += d
                    msg_s = apool.tile([P, W], F16, tag="msg_s")
                    nc.scalar.copy(msg_s[:], msg_ps[:])

                    # ---- MLP (feature-major, fp16 in / f32 accum) ----
                    xt = xtp[:, si * W:(si + 1) * W]

                    h_ps = pmlp.tile([P, W], F32, tag="mlp")
                    nc.tensor.matmul(h_ps[:], lhsT=w0x[:], rhs=xt,
                                     start=True, stop=False)
                    nc.tensor.matmul(h_ps[:], lhsT=w0m[:], rhs=msg_s[:],
                                     start=False, stop=True)
                    h1 = apool.tile([P, W], F16, tag="h")
                    nc.scalar.activation(h1[:], h_ps[:], AF.Relu,
                                         bias=b0[:, 0:1])

                    h_ps2 = pmlp.tile([P, W], F32, tag="mlp")
                    nc.tensor.matmul(h_ps2[:], lhsT=wh0[:], rhs=h1[:],
                                     start=True, stop=True)
                    h2 = apool.tile([P, W], F16, tag="h")
                    nc.scalar.activation(h2[:], h_ps2[:], AF.Relu,
                                         bias=bh0[:, 0:1])

                    h_ps3 = pmlp.tile([P, W], F32, tag="mlp")
                    nc.tensor.matmul(h_ps3[:], lhsT=wh1[:], rhs=h2[:],
                                     start=True, stop=True)
                    h3 = apool.tile([P, W], F16, tag="h")
                    nc.scalar.activation(h3[:], h_ps3[:], AF.Relu,
                                         bias=bh1[:, 0:1])

                    # ---- output layer, node-major out[node, ch] ----
                    o_ps = pout.tile([P, W], F32, tag="o")
                    for gi in range(SG):
                        nc.tensor.matmul(
                            o_ps[:, gi * P:(gi + 1) * P],
                            lhsT=h3[:, gi * P:(gi + 1) * P],
                            rhs=wo[:],
                            start=True, stop=True,
                        )

                    # ---- GroupNorm(1, C) + residual ----
                    basis = gpool.tile([P, W], F32, tag="basis")
                    if use_bo:
                        nc.vector.tensor_add(basis[:], o_ps[:], bo_b[:])
                    else:
                        nc.vector.tensor_copy(basis[:], o_ps[:])
                    b3 = basis[:].rearrange("p (g c) -> p g c", c=P)
                    s1 = tpool.tile([P, SG], F32, tag="s1")
                    nc.vector.tensor_reduce(s1[:], b3,
                                            axis=mybir.AxisListType.X,
                                            op=ALU.add)
                    s2 = tpool.tile([P, SG], F32, tag="s2")
                    sqf = gpool.tile([P, W], F32, tag="sqf")
                    nc.vector.tensor_tensor(sqf[:], basis[:], basis[:],
                                            op=ALU.mult)
                    nc.vector.tensor_reduce(
                        s2[:], sqf[:].rearrange("p (g c) -> p g c", c=P),
                        axis=mybir.AxisListType.X, op=ALU.add)
                    mu = tpool.tile([P, SG], F32, tag="mu")
                    nc.vector.tensor_scalar_mul(mu[:], s1[:], 1.0 / P)
                    ex2 = tpool.tile([P, SG], F32, tag="ex2")
                    nc.vector.tensor_scalar_mul(ex2[:], s2[:], 1.0 / P)
                    var = tpool.tile([P, SG], F32, tag="var")
                    nc.vector.tensor_tensor(var[:], mu[:], mu[:], op=ALU.mult)
                    nc.vector.tensor_tensor(var[:], ex2[:], var[:],
                                            op=ALU.subtract)
                    sd = tpool.tile([P, SG], F32, tag="sd")
                    nc.scalar.activation(sd[:], var[:], AF.Sqrt,
                                         bias=eps_t[:, 0:1])
                    rinv = tpool.tile([P, SG], F32, tag="rinv")
                    nc.vector.reciprocal(rinv[:], sd[:])
                    negms = tpool.tile([P, SG], F32, tag="negms")
                    nc.vector.tensor_tensor(negms[:], mu[:], rinv[:],
                                            op=ALU.mult)
                    nc.vector.tensor_scalar_mul(negms[:], negms[:], -1.0)

                    for gi in range(SG):
                        u = gpool.tile([P, P], F32, tag="u")
                        nc.scalar.activation(
                            u[:], basis[:, gi * P:(gi + 1) * P], AF.Identity,
                            bias=negms[:, gi:gi + 1],
                            scale=rinv[:, gi:gi + 1])
                        if use_gn:
                            nc.vector.tensor_tensor(
                                u[:], u[:], gnw_b[:, gi * P:(gi + 1) * P],
                                op=ALU.mult)
                            nc.vector.tensor_tensor(
                                u[:], u[:], gnb_b[:, gi * P:(gi + 1) * P],
                                op=ALU.add)
                        off = si * W + gi * P
                        nc.vector.tensor_add(otp[:, off:off + P], u[:],
                                             xres[:, off:off + P])
                dma_eng.dma_start(out_d[:, s0 * W:s0 * W + wn], otp[:])

    return nc


# --------------------------------------------------------------------------
# Entry point
# --------------------------------------------------------------------------

def _run(inputs, trace=False):
    x = np.asarray(inputs["x"], np.float32)
    e = np.asarray(inputs["e"], np.float32)
    edge_index = np.asarray(inputs["edge_index"])
    W0 = np.asarray(inputs["W0"], np.float32)
    b0 = np.asarray(inputs["b0"], np.float32)
    Wh = np.asarray(inputs["Wh"], np.float32)
    bh = np.asarray(inputs["bh"], np.float32)
    Wo = np.asarray(inputs["Wo"], np.float32)
    bo = np.asarray(inputs["bo"], np.float32)
    gn_w = np.asarray(inputs["gn_w"], np.float32)
    gn_b = np.asarray(inputs["gn_b"], np.float32)

    import time as _time
    _t0 = _time.monotonic()
    in_maps, meta = _host_prep(x, e, edge_index)
    xbufs = _host_prep_x(x, meta)
    print(f"[kernel] host prep {_time.monotonic()-_t0:.1f}s", flush=True)

    flags = dict(use_bo=bool(np.any(bo != 0.0)),
                 use_gn=bool(np.any(gn_w != 1.0) or np.any(gn_b != 0.0)))

    consts = dict(
        W0x=np.ascontiguousarray(W0[:P], np.float16),
        W0m=np.ascontiguousarray(W0[P:], np.float16),
        Wh0=np.ascontiguousarray(Wh[0], np.float16),
        Wh1=np.ascontiguousarray(Wh[1], np.float16),
        Wo=np.ascontiguousarray(Wo, np.float16),
        b0=b0.reshape(P, 1).copy(),
        bh0=bh[0].reshape(P, 1).copy(),
        bh1=bh[1].reshape(P, 1).copy(),
        I16=np.eye(P, dtype=np.float16),
    )
    if flags["use_bo"]:
        consts["bo_b"] = np.tile(bo[None, :], (P, SG)).astype(np.float32)
    if flags["use_gn"]:
        consts["gnw_b"] = np.tile(gn_w[None, :], (P, SG)).astype(np.float32)
        consts["gnb_b"] = np.tile(gn_b[None, :], (P, SG)).astype(np.float32)

    _t0 = _time.monotonic()
    nc = _build_program(meta, flags)
    print(f"[kernel] build {_time.monotonic()-_t0:.1f}s", flush=True)
    _t0 = _time.monotonic()
    nc.compile()
    print(f"[kernel] bacc compile {_time.monotonic()-_t0:.1f}s", flush=True)
    _t0 = _time.monotonic()

    full_maps = []
    for c in range(N_CORES):
        m = dict(in_maps[c])
        m["x_pm"], m["xT"] = xbufs[c]
        m.update(consts)
        full_maps.append(m)

    res = run_bass_kernel_spmd(nc, full_maps, list(range(N_CORES)),
                               trace=trace)
    print(f"[kernel] spmd run {_time.monotonic()-_t0:.1f}s", flush=True)

    n_nodes = x.shape[0]
    out = np.empty((n_nodes, P), np.float32)
    ngrp = meta["ngrp"]
    for c, cinfo in enumerate(meta["cores"]):
        oc = np.asarray(res.results[c]["out"])          # [P, ngrp*P]
        on = oc.reshape(P, ngrp, P).transpose(1, 0, 2).reshape(ngrp * P, P)
        perm = cinfo["order"]
        valid = perm < cinfo["n_real"]
        rows = np.nonzero(valid)[0]
        out[cinfo["lo"] + perm[valid]] = on[rows]
    return out, res


def kernel(**inputs):
    out, _ = _run(inputs, trace=False)
    return out
